# revision 3
# baseline (speedup 1.0000x reference)
"""Causal multi-head self-attention (b=4, s=2048, d_model=1024, 16 heads) on 8
Trainium2 NeuronCores.

Sharding: core c handles batch c//2 and head-group c%2 (8 of 16 heads):
  - wqkv row-split by head (tensor parallel), wo column-split by head.
  - Each core returns the partial output projection [s, d_model] for its head
    group; the host sums the two partials of each batch while unsharding (the
    pairwise all-reduce of the TP split).

v2: t-chunk-major software pipeline. The kernel runs 4 stages, one per
512-token chunk. Each stage does, for its chunk t:
  A) Q/K projection + fused RoPE for all 4 head pairs (PSUM -> bf16 CAST,
     two 2x-mode bf16 tensor_tensor multiplies with bf16 cos/sin tables,
     SWDGE swap-add DMA for the rotate-half),
  B) V projection for the chunk (66-stride vbuf layout so the PSUM->SBUF
     CAST runs in 2x mode; ones column at 64 for free softmax denominators),
  C) causal attention for q-chunk t against all k-tiles <= t for all 4 head
     pairs (row-tiled score matmuls K=64x2, mask via identity@mtri accumulate,
     exp on ScalarE with column trimming, AV with M=65 ones-column lhsT,
     denominator broadcast via K=2 block-diag ones matmul + DVE reciprocal),
  D) output projection for the chunk's rows (contraction over all 4 head
     pairs' yT), bf16 staging, DMA out.
This keeps ScalarE's exp stream (the #2 engine, ~150us) overlapped with
TensorE work for the whole kernel instead of only the middle, starts matmuls
within ~5us of launch, and eliminates the phase-B/D serial head and tail of
v1. x is loaded once per stage (v1 loaded it twice).
"""

import sys

if "/opt/trn_rl_repo" not in sys.path:
    sys.path.insert(0, "/opt/trn_rl_repo")

from contextlib import ExitStack

import numpy as np

import concourse.bass as bass  # noqa: F401
import concourse.tile as tile
from concourse import bacc, mybir
from concourse.bass_utils import run_bass_kernel_spmd

F32 = mybir.dt.float32
F32R = mybir.dt.float32r
BF16 = mybir.dt.bfloat16
EXP = mybir.ActivationFunctionType.Exp
MULT = mybir.AluOpType.mult
ADD = mybir.AluOpType.add

# Problem constants
B, S_FULL, D = 4, 2048, 1024
NH_CORE = 8      # heads per core
DH = 64          # head dim
FQK = 1024       # Q+K features per core
FV = 512         # V features per core
P = 128
TCH = 512        # q/t chunk size
VST = 66         # vbuf per-head stride (64 V dims + ones col + pad for 4B align)
NEG = -1.0e30
ROPE_THETA = 10000.0
SCALE = 1.0 / 8.0  # 1/sqrt(DH)

_CACHE = {}


def _emit(nc, tc, S, xT, wqkT, wvT, woT, cosF, sinFpm, mtri, ident, ones2, onesv, outp):
    n_st = S // TCH          # stages
    n_hp = NH_CORE // 2
    mm = nc.tensor.matmul

    xT_r = xT.ap().rearrange("(eo p) t -> p eo t", p=P)
    wqk_r = wqkT.ap().rearrange("(eo p) f -> p eo f", p=P)
    wv_r = wvT.ap().rearrange("(eo p) f -> p eo f", p=P)
    wo_r = woT.ap().rearrange("(co p) j -> p co j", p=P)

    with ExitStack() as ctx:
        # ---------- persistent SBUF ----------
        persist = ctx.enter_context(tc.tile_pool(name="persist", bufs=1))
        wqk_sb = persist.tile([P, 8, FQK], BF16, tag="wqk", name="wqk_sb")
        wv_sb = persist.tile([P, 8, FV], BF16, tag="wv", name="wv_sb")
        wo_sb = persist.tile([P, 4, D], BF16, tag="wo", name="wo_sb")
        qkT = [persist.tile([P, S], BF16, tag=f"qkT{ft}", name=f"qkT{ft}") for ft in range(8)]
        vbuf = persist.tile([P, S // P, NH_CORE, VST], BF16, tag="vbuf", name="vbuf")
        yT = [persist.tile([P, S], BF16, tag=f"yT{hp}", name=f"yT{hp}") for hp in range(n_hp)]
        ident_sb = persist.tile([P, P], BF16, tag="ident", name="ident_sb")
        mtri_sb = persist.tile([P, P], BF16, tag="mtri", name="mtri_sb")
        ones2_sb = persist.tile([2, P], F32R, tag="ones2", name="ones2_sb")

        # streamed per-stage inputs
        xpool = ctx.enter_context(tc.tile_pool(name="xchunk", bufs=2))
        cspool = ctx.enter_context(tc.tile_pool(name="costab", bufs=2))
        bfpool = ctx.enter_context(tc.tile_pool(name="qkbf", bufs=3))
        btpool = ctx.enter_context(tc.tile_pool(name="btmp", bufs=3))
        epool = ctx.enter_context(tc.tile_pool(name="expS", bufs=5))
        dpool = ctx.enter_context(tc.tile_pool(name="denst", bufs=2))
        rpool = ctx.enter_context(tc.tile_pool(name="recb", bufs=2))
        otpool = ctx.enter_context(tc.tile_pool(name="outsb", bufs=3))

        # PSUM: 4 + 2 + 2 = 8 banks
        s_ps = ctx.enter_context(tc.tile_pool(name="s_psum", bufs=2, space="PSUM"))
        av_ps = ctx.enter_context(tc.tile_pool(name="av_psum", bufs=1, space="PSUM"))
        m_ps = ctx.enter_context(tc.tile_pool(name="m_psum", bufs=2, space="PSUM"))

        # ---------- initial loads ----------
        nc.sync.dma_start(ident_sb[:], ident.ap()[:, :])
        nc.sync.dma_start(mtri_sb[:], mtri.ap()[:, :])
        nc.sync.dma_start(ones2_sb[:], ones2.ap()[:, :])
        nc.sync.dma_start(vbuf[:, :, :, DH : DH + 1], onesv.ap()[:, :, :, :])
        # weights: interleave across the two HWDGE queues; ec-sliced so the
        # first projection matmuls can start after the first slices land
        for ec in range(8):
            eng = nc.sync if ec % 2 == 0 else nc.scalar
            eng.dma_start(wqk_sb[:, ec, :], wqk_r[:, ec, :])
        for ec in range(8):
            eng = nc.scalar if ec % 2 == 0 else nc.sync
            eng.dma_start(wv_sb[:, ec, :], wv_r[:, ec, :])
        for cc in range(4):
            nc.scalar.dma_start(wo_sb[:, cc, :], wo_r[:, cc, :])

        def load_stage_inputs(s):
            tsl = slice(s * TCH, (s + 1) * TCH)
            xch = xpool.tile([P, 8, TCH], BF16, tag="xch", name="xch")
            nc.sync.dma_start(xch[:], xT_r[:, :, tsl])
            cos_ch = cspool.tile([P, TCH], BF16, tag="cos", name="cos")
            sin_ch = cspool.tile([P, TCH], BF16, tag="sin", name="sin")
            nc.sync.dma_start(cos_ch[:], cosF.ap()[:, tsl])
            nc.sync.dma_start(sin_ch[:], sinFpm.ap()[:, tsl])
            return xch, cos_ch, sin_ch

        xin = load_stage_inputs(0)

        for s in range(n_st):
            tsl = slice(s * TCH, (s + 1) * TCH)
            xch, cos_ch, sin_ch = xin
            if s + 1 < n_st:
                xin = load_stage_inputs(s + 1)

            # ---- A: Q/K projection + RoPE for this chunk ----
            for hp in range(n_hp):
                for ft in (hp, 4 + hp):
                    qkps = m_ps.tile([P, TCH], F32, tag="m", name="qkps")
                    for ec in range(8):
                        mm(
                            qkps[:],
                            wqk_sb[:, ec, ft * P : (ft + 1) * P],
                            xch[:, ec, :],
                            start=(ec == 0),
                            stop=(ec == 7),
                        )
                    qk_bf = bfpool.tile([P, TCH], BF16, tag="qkbf", name="qkbf")
                    nc.vector.tensor_copy(qk_bf[:], qkps[:])
                    nc.vector.tensor_tensor(qkT[ft][:, tsl], qk_bf[:], cos_ch[:], MULT)
                    bt = btpool.tile([P, TCH], BF16, tag="bt", name="bt")
                    nc.vector.tensor_tensor(bt[:], qk_bf[:], sin_ch[:], MULT)
                    for blk in range(4):
                        a = blk * 32
                        c2 = a ^ 32
                        nc.gpsimd.dma_start(
                            qkT[ft][c2 : c2 + 32, tsl], bt[a : a + 32, :], accum_op=ADD
                        )

            # ---- B: V projection for this chunk ----
            for tti in range(TCH // P):
                kt = s * (TCH // P) + tti
                vps = m_ps.tile([P, FV], F32, tag="m", name="vps")
                for ec in range(8):
                    mm(
                        vps[:],
                        xch[:, ec, tti * P : (tti + 1) * P],
                        wv_sb[:, ec, :],
                        start=(ec == 0),
                        stop=(ec == 7),
                    )
                nc.vector.tensor_copy(vbuf[:, kt, :, 0:DH], vps[:])

            # ---- C: attention, q-chunk s vs k-tiles 0..4s+3 ----
            nkt = (TCH // P) * (s + 1)
            for hp in range(n_hp):
                qt = qkT[hp]
                ktt = qkT[4 + hp]
                h0, h1 = 2 * hp, 2 * hp + 1
                avp = av_ps.tile([DH + 1, 2 * TCH], F32, tag="avp", name="avp")
                for ki in range(nkt):
                    ksl = slice(ki * P, (ki + 1) * P)
                    diag = ki >= (TCH // P) * s
                    j = ki - (TCH // P) * s
                    off = j * P if diag else 0
                    qsl = slice(s * TCH + off, (s + 1) * TCH)
                    sp = s_ps.tile([P, 2 * TCH], F32, tag="sp", name="sp")
                    mm(
                        sp[:, off:TCH],
                        ktt[0:64, ksl],
                        qt[0:64, qsl],
                        start=True,
                        stop=True,
                    )
                    mm(
                        sp[:, TCH + off : 2 * TCH],
                        ktt[64:128, ksl],
                        qt[64:128, qsl],
                        start=True,
                        stop=True,
                    )
                    if diag:
                        mm(
                            sp[:, off : off + P],
                            ident_sb[:],
                            mtri_sb[:],
                            start=False,
                            stop=True,
                            skip_group_check=True,
                        )
                        mm(
                            sp[:, TCH + off : TCH + off + P],
                            ident_sb[:],
                            mtri_sb[:],
                            start=False,
                            stop=True,
                            skip_group_check=True,
                        )
                    e = epool.tile([P, 2 * TCH], BF16, tag="e", name="e")
                    sp3 = sp[:].rearrange("p (h q) -> p h q", h=2)
                    e3 = e[:].rearrange("p (h q) -> p h q", h=2)
                    nc.scalar.activation(
                        e3[:, :, off:], sp3[:, :, off:], EXP, scale=SCALE
                    )
                    mm(
                        avp[:, off:TCH],
                        vbuf[:, ki, h0, 0 : DH + 1],
                        e[:, off:TCH],
                        start=(ki == 0),
                        stop=(ki == nkt - 1),
                        skip_group_check=True,
                    )
                    mm(
                        avp[:, TCH + off : 2 * TCH],
                        vbuf[:, ki, h1, 0 : DH + 1],
                        e[:, TCH + off : 2 * TCH],
                        start=(ki == 0),
                        stop=(ki == nkt - 1),
                        skip_group_check=True,
                    )
                # denominators (avp row 64) -> [2, 512] f32r -> K=2 ones matmul
                denf = dpool.tile([1, 2 * TCH], F32R, tag="denf", name="denf")
                nc.vector.tensor_copy(denf[:], avp[DH : DH + 1, :])
                den2 = dpool.tile([2, TCH], F32R, tag="den2", name="den2")
                nc.sync.dma_start(den2[0:1, :], denf[0:1, 0:TCH])
                nc.sync.dma_start(den2[1:2, :], denf[0:1, TCH : 2 * TCH])
                rb = m_ps.tile([P, TCH], F32, tag="m", name="rb")
                mm(rb[:], ones2_sb[:, :], den2[:], start=True, stop=True)
                rec = rpool.tile([P, TCH], F32, tag="rec", name="rec")
                rscr = rpool.tile([P, TCH], F32, tag="rscr", name="rscr")
                nc.vector.reciprocal_approx_accurate(rec[:], rb[:], rscr[:])
                nc.vector.tensor_tensor(
                    yT[hp][0:64, tsl], avp[0:DH, 0:TCH], rec[0:64, :], MULT
                )
                nc.vector.tensor_tensor(
                    yT[hp][64:128, tsl],
                    avp[0:DH, TCH : 2 * TCH],
                    rec[64:128, :],
                    MULT,
                )

            # ---- D: output projection for this chunk's rows ----
            for tti in range(TCH // P):
                ttsl = slice(s * TCH + tti * P, s * TCH + (tti + 1) * P)
                for jc in range(2):
                    jsl = slice(jc * TCH, (jc + 1) * TCH)
                    op = m_ps.tile([P, TCH], F32, tag="m", name="op")
                    for cc in range(4):
                        mm(
                            op[:],
                            yT[cc][:, ttsl],
                            wo_sb[:, cc, jsl],
                            start=(cc == 0),
                            stop=(cc == 3),
                        )
                    ot = otpool.tile([P, TCH], BF16, tag="ot", name="ot")
                    nc.vector.tensor_copy(ot[:], op[:])
                    nc.sync.dma_start(outp.ap()[ttsl, jsl], ot[:])


def _build(S=S_FULL):
    key = ("nc", S)
    if key in _CACHE:
        return _CACHE[key]
    nc = bacc.Bacc("TRN2", target_bir_lowering=False, debug=False, num_devices=8)
    xT = nc.dram_tensor("xT", [D, S], BF16, kind="ExternalInput")
    wqkT = nc.dram_tensor("wqkT", [D, FQK], BF16, kind="ExternalInput")
    wvT = nc.dram_tensor("wvT", [D, FV], BF16, kind="ExternalInput")
    woT = nc.dram_tensor("woT", [FV, D], BF16, kind="ExternalInput")
    cosF = nc.dram_tensor("cosF", [P, S], BF16, kind="ExternalInput")
    sinFpm = nc.dram_tensor("sinFpm", [P, S], BF16, kind="ExternalInput")
    mtri = nc.dram_tensor("mtri", [P, P], BF16, kind="ExternalInput")
    ident = nc.dram_tensor("ident", [P, P], BF16, kind="ExternalInput")
    ones2 = nc.dram_tensor("ones2", [2, P], F32R, kind="ExternalInput")
    onesv = nc.dram_tensor(
        "onesv", [P, S // P, NH_CORE, 1], BF16, kind="ExternalInput"
    )
    outp = nc.dram_tensor("outp", [S, D], BF16, kind="ExternalOutput")
    with tile.TileContext(nc) as tc:
        _emit(nc, tc, S, xT, wqkT, wvT, woT, cosF, sinFpm, mtri, ident, ones2, onesv, outp)
    nc.compile()
    _CACHE[key] = nc
    return nc


def host_inputs(x, wqkv, wo, token_positions, S=S_FULL):
    """Build the 8 per-core input maps (host-side sharding / layout prep)."""
    x = np.asarray(x, dtype=np.float32)
    wqkv = np.asarray(wqkv, dtype=np.float32)
    wo = np.asarray(wo, dtype=np.float32)
    pos = np.asarray(token_positions).astype(np.float32)

    d_model = x.shape[2]
    wq, wk, wv = wqkv[0:d_model], wqkv[d_model : 2 * d_model], wqkv[2 * d_model :]

    inv = np.float32(ROPE_THETA) ** (
        -np.arange(0, DH, 2, dtype=np.float32) / np.float32(DH)
    )  # [32]
    ang = pos[None, :] * inv[:, None]  # [32, S]
    cos32 = np.cos(ang).astype(np.float32)
    sin32 = np.sin(ang).astype(np.float32)

    import ml_dtypes

    cosF = np.tile(cos32, (4, 1)).astype(ml_dtypes.bfloat16)  # [128, S]
    sinFpm = np.tile(np.concatenate([sin32, -sin32], axis=0), (2, 1)).astype(
        ml_dtypes.bfloat16
    )  # [128, S]

    a = np.arange(P)
    mtri = np.where(a[:, None] > a[None, :], np.float32(NEG), np.float32(0.0))
    mtri = mtri.astype(ml_dtypes.bfloat16)
    ident = np.eye(P, dtype=ml_dtypes.bfloat16)
    S = x.shape[1]
    ones2 = np.zeros((2, P), np.float32)
    ones2[0, 0:64] = 1.0
    ones2[1, 64:128] = 1.0
    onesv = np.ones((P, S // P, NH_CORE, 1), ml_dtypes.bfloat16)

    perm64 = np.concatenate([np.arange(0, DH, 2), np.arange(1, DH, 2)])

    in_maps = []
    for ci in range(8):
        bi, hg = divmod(ci, 2)
        xT = np.ascontiguousarray(x[bi].T)
        rows = []
        for blk in (wq, wk):
            for h in range(hg * NH_CORE, (hg + 1) * NH_CORE):
                rows.append(blk[h * DH : (h + 1) * DH][perm64])
        wqkT = np.ascontiguousarray(np.concatenate(rows, axis=0).T)
        wvT = np.ascontiguousarray(wv[hg * FV : (hg + 1) * FV].T)
        woT = np.ascontiguousarray(wo[:, hg * FV : (hg + 1) * FV].T)
        xT = xT.astype(ml_dtypes.bfloat16)
        wqkT = wqkT.astype(ml_dtypes.bfloat16)
        wvT = wvT.astype(ml_dtypes.bfloat16)
        woT = woT.astype(ml_dtypes.bfloat16)
        in_maps.append(
            {
                "xT": xT,
                "wqkT": wqkT,
                "wvT": wvT,
                "woT": woT,
                "cosF": cosF,
                "sinFpm": sinFpm,
                "mtri": mtri,
                "ident": ident,
                "ones2": ones2,
                "onesv": onesv,
            }
        )
    return in_maps


def _install_ntff_hook():
    """Recreate the antenv.axon_hooks NTFF profile hook this image lacks
    (same ctypes shim trn_agent_boot would register). Dev/profiling only."""
    import contextlib
    import ctypes
    import os
    import types

    try:
        import antenv.axon_hooks  # noqa: F401

        return
    except ImportError:
        pass
    so_path = "/opt/axon/libaxon_pjrt.so"
    if not os.path.exists(so_path):
        return
    lib = ctypes.CDLL(so_path)
    if not hasattr(lib, "axon_start_nrt_profile"):
        return
    lib.axon_start_nrt_profile.argtypes = [
        ctypes.POINTER(ctypes.c_int64),
        ctypes.c_size_t,
    ]
    lib.axon_start_nrt_profile.restype = ctypes.c_int64
    lib.axon_stop_nrt_profile.argtypes = [ctypes.c_char_p]
    lib.axon_stop_nrt_profile.restype = ctypes.c_int64

    @contextlib.contextmanager
    def _hook(output_dir, device_ids):
        import jax

        jax.devices()
        if device_ids:
            ids = (ctypes.c_int64 * len(device_ids))(*device_ids)
            rc = lib.axon_start_nrt_profile(ids, len(device_ids))
        else:
            rc = lib.axon_start_nrt_profile(None, 0)
        if rc != 0:
            raise RuntimeError(f"axon_start_nrt_profile rc={rc}")
        try:
            yield
        finally:
            n = lib.axon_stop_nrt_profile(str(output_dir).encode())
            if n < 0:
                raise RuntimeError(f"axon_stop_nrt_profile rc={n}")

    import antenv
    from concourse import bass_utils as _bu

    _bu.upload_artifacts = lambda d: d  # no bucket access in this container
    mod = types.ModuleType("antenv.axon_hooks")
    mod.get_axon_ntff_profile_hook = lambda: _hook
    mod.set_axon_ntff_profile_hook = lambda h: None
    sys.modules["antenv.axon_hooks"] = mod
    antenv.axon_hooks = mod


def kernel(x, wqkv, wo, token_positions, trace=False):
    if trace:
        _install_ntff_hook()
    nc = _build()
    in_maps = host_inputs(x, wqkv, wo, token_positions)
    res = run_bass_kernel_spmd(nc, in_maps, core_ids=list(range(8)), trace=trace)
    parts = [np.asarray(res.results[ci]["outp"], dtype=np.float32) for ci in range(8)]
    out = np.stack([parts[2 * bi] + parts[2 * bi + 1] for bi in range(B)], axis=0)
    if trace:
        kernel.last_result = res
    return out


# revision 8
# speedup vs baseline: 1.1516x; 1.1516x over previous
"""Causal multi-head self-attention (b=4, s=2048, d_model=1024, 16 heads) on 8
Trainium2 NeuronCores.

Sharding: core c handles batch c//2 and head-group c%2 (8 of 16 heads):
  - wqkv row-split by head (tensor parallel), wo column-split by head.
  - Each core returns the partial output projection [s, d_model] for its head
    group; the host sums the two partials of each batch while unsharding (the
    pairwise all-reduce of the TP split).

v2: t-chunk-major software pipeline. The kernel runs 4 stages, one per
512-token chunk. Each stage does, for its chunk t:
  A) Q/K projection + fused RoPE for all 4 head pairs (PSUM -> bf16 CAST,
     two 2x-mode bf16 tensor_tensor multiplies with bf16 cos/sin tables,
     SWDGE swap-add DMA for the rotate-half),
  B) V projection for the chunk (66-stride vbuf layout so the PSUM->SBUF
     CAST runs in 2x mode; ones column at 64 for free softmax denominators),
  C) causal attention for q-chunk t against all k-tiles <= t for all 4 head
     pairs (row-tiled score matmuls K=64x2, mask via identity@mtri accumulate,
     exp on ScalarE with column trimming, AV with M=65 ones-column lhsT,
     denominator broadcast via K=2 block-diag ones matmul + DVE reciprocal),
  D) output projection for the chunk's rows (contraction over all 4 head
     pairs' yT), bf16 staging, DMA out.
This keeps ScalarE's exp stream (the #2 engine, ~150us) overlapped with
TensorE work for the whole kernel instead of only the middle, starts matmuls
within ~5us of launch, and eliminates the phase-B/D serial head and tail of
v1. x is loaded once per stage (v1 loaded it twice).
"""

import sys

if "/opt/trn_rl_repo" not in sys.path:
    sys.path.insert(0, "/opt/trn_rl_repo")

from contextlib import ExitStack

import numpy as np

import concourse.bass as bass  # noqa: F401
import concourse.tile as tile
from concourse import bacc, mybir
from concourse.bass_utils import run_bass_kernel_spmd

F32 = mybir.dt.float32
F32R = mybir.dt.float32r
BF16 = mybir.dt.bfloat16
EXP = mybir.ActivationFunctionType.Exp
MULT = mybir.AluOpType.mult
ADD = mybir.AluOpType.add

# Problem constants
B, S_FULL, D = 4, 2048, 1024
NH_CORE = 8      # heads per core
DH = 64          # head dim
FQK = 1024       # Q+K features per core
FV = 512         # V features per core
P = 128
TCH = 512        # q/t chunk size
VST = 66         # vbuf per-head stride (64 V dims + ones col + pad for 4B align)
NEG = -1.0e30
ROPE_THETA = 10000.0
SCALE = 1.0 / 8.0  # 1/sqrt(DH)

_CACHE = {}


def _emit(nc, tc, S, xT, wqkT, wvT, woT, cosF, sinFpm, mtri, ident, ones2, onesv, outp):
    n_st = S // TCH          # stages
    n_hp = NH_CORE // 2
    mm = nc.tensor.matmul

    xT_r = xT.ap().rearrange("(eo p) t -> p eo t", p=P)
    wqk_r = wqkT.ap().rearrange("(eo p) f -> p eo f", p=P)
    wv_r = wvT.ap().rearrange("(eo p) f -> p eo f", p=P)
    wo_r = woT.ap().rearrange("(co p) j -> p co j", p=P)

    with ExitStack() as ctx:
        # ---------- persistent SBUF ----------
        persist = ctx.enter_context(tc.tile_pool(name="persist", bufs=1))
        wqk_sb = persist.tile([P, 8, FQK], BF16, tag="wqk", name="wqk_sb")
        wv_sb = persist.tile([P, 8, FV], BF16, tag="wv", name="wv_sb")
        wo_sb = persist.tile([P, 4, D], BF16, tag="wo", name="wo_sb")
        qkT = [persist.tile([P, S], BF16, tag=f"qkT{ft}", name=f"qkT{ft}") for ft in range(8)]
        vbuf = persist.tile([P, S // P, NH_CORE, VST], BF16, tag="vbuf", name="vbuf")
        yT = [persist.tile([P, S], BF16, tag=f"yT{hp}", name=f"yT{hp}") for hp in range(n_hp)]
        ident_sb = persist.tile([P, P], BF16, tag="ident", name="ident_sb")
        mtri_sb = persist.tile([P, P], BF16, tag="mtri", name="mtri_sb")
        ones2_sb = persist.tile([1, 2 * P], F32R, tag="ones2", name="ones2_sb")

        # streamed per-stage inputs
        xpool = ctx.enter_context(tc.tile_pool(name="xchunk", bufs=2))
        cspool = ctx.enter_context(tc.tile_pool(name="costab", bufs=2))
        bfpool = ctx.enter_context(tc.tile_pool(name="qkbf", bufs=3))
        btpool = ctx.enter_context(tc.tile_pool(name="btmp", bufs=3))
        epool = ctx.enter_context(tc.tile_pool(name="expS", bufs=5))
        dpool = ctx.enter_context(tc.tile_pool(name="denst", bufs=2))
        rpool = ctx.enter_context(tc.tile_pool(name="recb", bufs=2))
        otpool = ctx.enter_context(tc.tile_pool(name="outsb", bufs=3))

        # PSUM: 4 + 2 + 2 = 8 banks
        s_ps = ctx.enter_context(tc.tile_pool(name="s_psum", bufs=2, space="PSUM"))
        av_ps = ctx.enter_context(tc.tile_pool(name="av_psum", bufs=1, space="PSUM"))
        m_ps = ctx.enter_context(tc.tile_pool(name="m_psum", bufs=2, space="PSUM"))

        # ---------- initial loads ----------
        nc.sync.dma_start(ident_sb[:], ident.ap()[:, :])
        nc.sync.dma_start(mtri_sb[:], mtri.ap()[:, :])
        nc.sync.dma_start(ones2_sb[:], ones2.ap()[:, :])
        nc.sync.dma_start(vbuf[:, :, :, DH : DH + 1], onesv.ap()[:, :, :, :])

        def load_stage_inputs(s):
            tsl = slice(s * TCH, (s + 1) * TCH)
            xch = xpool.tile([P, 8, TCH], BF16, tag="xch", name="xch")
            nc.sync.dma_start(xch[:], xT_r[:, :, tsl])
            cos_ch = cspool.tile([P, TCH], BF16, tag="cos", name="cos")
            sin_ch = cspool.tile([P, TCH], BF16, tag="sin", name="sin")
            nc.sync.dma_start(cos_ch[:], cosF.ap()[:, tsl])
            nc.sync.dma_start(sin_ch[:], sinFpm.ap()[:, tsl])
            return xch, cos_ch, sin_ch

        xin = load_stage_inputs(0)

        # weights after the stage-0 activations: interleave across the two
        # HWDGE queues; ec-sliced so the first matmuls start after slice 0
        for ec in range(8):
            eng = nc.sync if ec % 2 == 0 else nc.scalar
            eng.dma_start(wqk_sb[:, ec, :], wqk_r[:, ec, :])
        for ec in range(8):
            eng = nc.scalar if ec % 2 == 0 else nc.sync
            eng.dma_start(wv_sb[:, ec, :], wv_r[:, ec, :])
        for cc in range(4):
            nc.scalar.dma_start(wo_sb[:, cc, :], wo_r[:, cc, :])

        for s in range(n_st):
            tsl = slice(s * TCH, (s + 1) * TCH)
            xch, cos_ch, sin_ch = xin
            if s + 1 < n_st:
                xin = load_stage_inputs(s + 1)

            # ---- A: Q/K projection + RoPE for this chunk ----
            for hp in range(n_hp):
                for ft in (hp, 4 + hp):
                    qkps = m_ps.tile([P, TCH], F32, tag="m", name="qkps")
                    for ec in range(8):
                        mm(
                            qkps[:],
                            wqk_sb[:, ec, ft * P : (ft + 1) * P],
                            xch[:, ec, :],
                            start=(ec == 0),
                            stop=(ec == 7),
                        )
                    qk_bf = bfpool.tile([P, TCH], BF16, tag="qkbf", name="qkbf")
                    nc.vector.tensor_copy(qk_bf[:], qkps[:])
                    cp = bfpool.tile([P, TCH], BF16, tag="cp", name="cp")
                    nc.vector.tensor_tensor(cp[:], qk_bf[:], cos_ch[:], MULT)
                    bt = btpool.tile([P, TCH], BF16, tag="bt", name="bt")
                    nc.vector.tensor_tensor(bt[:], qk_bf[:], sin_ch[:], MULT)
                    # rotate-half: HWDGE 32-row swapped copy, then one 2x bf16 add
                    bs = btpool.tile([P, TCH], BF16, tag="bs", name="bs")
                    for blk in range(4):
                        a = blk * 32
                        c2 = a ^ 32
                        nc.sync.dma_start(bs[c2 : c2 + 32, :], bt[a : a + 32, :])
                    nc.vector.tensor_tensor(qkT[ft][:, tsl], cp[:], bs[:], ADD)

            # ---- B: V projection for this chunk ----
            for tti in range(TCH // P):
                kt = s * (TCH // P) + tti
                vps = m_ps.tile([P, FV], F32, tag="m", name="vps")
                for ec in range(8):
                    mm(
                        vps[:],
                        xch[:, ec, tti * P : (tti + 1) * P],
                        wv_sb[:, ec, :],
                        start=(ec == 0),
                        stop=(ec == 7),
                    )
                nc.vector.tensor_copy(vbuf[:, kt, :, 0:DH], vps[:])

            # ---- C: attention, q-chunk s vs k-tiles 0..4s+3 ----
            nkt = (TCH // P) * (s + 1)
            for hp in range(n_hp):
                qt = qkT[hp]
                ktt = qkT[4 + hp]
                h0, h1 = 2 * hp, 2 * hp + 1
                avp = av_ps.tile([DH + 1, 2 * TCH], F32, tag="avp", name="avp")
                for ki in range(nkt):
                    ksl = slice(ki * P, (ki + 1) * P)
                    diag = ki >= (TCH // P) * s
                    j = ki - (TCH // P) * s
                    off = j * P if diag else 0
                    qsl = slice(s * TCH + off, (s + 1) * TCH)
                    sp = s_ps.tile([P, 2 * TCH], F32, tag="sp", name="sp")
                    mm(
                        sp[:, off:TCH],
                        ktt[0:64, ksl],
                        qt[0:64, qsl],
                        start=True,
                        stop=True,
                    )
                    mm(
                        sp[:, TCH + off : 2 * TCH],
                        ktt[64:128, ksl],
                        qt[64:128, qsl],
                        start=True,
                        stop=True,
                    )
                    if diag:
                        mm(
                            sp[:, off : off + P],
                            ident_sb[:],
                            mtri_sb[:],
                            start=False,
                            stop=True,
                            skip_group_check=True,
                        )
                        mm(
                            sp[:, TCH + off : TCH + off + P],
                            ident_sb[:],
                            mtri_sb[:],
                            start=False,
                            stop=True,
                            skip_group_check=True,
                        )
                    e = epool.tile([P, 2 * TCH], BF16, tag="e", name="e")
                    sp3 = sp[:].rearrange("p (h q) -> p h q", h=2)
                    e3 = e[:].rearrange("p (h q) -> p h q", h=2)
                    nc.scalar.activation(
                        e3[:, :, off:], sp3[:, :, off:], EXP, scale=SCALE
                    )
                    mm(
                        avp[:, off:TCH],
                        vbuf[:, ki, h0, 0 : DH + 1],
                        e[:, off:TCH],
                        start=(ki == 0),
                        stop=(ki == nkt - 1),
                        skip_group_check=True,
                    )
                    mm(
                        avp[:, TCH + off : 2 * TCH],
                        vbuf[:, ki, h1, 0 : DH + 1],
                        e[:, TCH + off : 2 * TCH],
                        start=(ki == 0),
                        stop=(ki == nkt - 1),
                        skip_group_check=True,
                    )
                # denominators (avp row 64) -> two K=1 masked matmuls broadcast
                # head0's denoms to partitions 0:64 and head1's to 64:128
                denf = dpool.tile([1, 2 * TCH], F32R, tag="denf", name="denf")
                nc.vector.tensor_copy(denf[:], avp[DH : DH + 1, :])
                rb = m_ps.tile([P, TCH], F32, tag="m", name="rb")
                mm(
                    rb[:],
                    ones2_sb[0:1, 0:P],
                    denf[0:1, 0:TCH],
                    start=True,
                    stop=False,
                )
                mm(
                    rb[:],
                    ones2_sb[0:1, P : 2 * P],
                    denf[0:1, TCH : 2 * TCH],
                    start=False,
                    stop=True,
                )
                rec = rpool.tile([P, TCH], F32, tag="rec", name="rec")
                rscr = rpool.tile([P, TCH], F32, tag="rscr", name="rscr")
                nc.vector.reciprocal_approx_accurate(rec[:], rb[:], rscr[:])
                nc.vector.tensor_tensor(
                    yT[hp][0:64, tsl], avp[0:DH, 0:TCH], rec[0:64, :], MULT
                )
                nc.vector.tensor_tensor(
                    yT[hp][64:128, tsl],
                    avp[0:DH, TCH : 2 * TCH],
                    rec[64:128, :],
                    MULT,
                )

            # ---- D: output projection for this chunk's rows ----
            for tti in range(TCH // P):
                ttsl = slice(s * TCH + tti * P, s * TCH + (tti + 1) * P)
                for jc in range(2):
                    jsl = slice(jc * TCH, (jc + 1) * TCH)
                    op = m_ps.tile([P, TCH], F32, tag="m", name="op")
                    for cc in range(4):
                        mm(
                            op[:],
                            yT[cc][:, ttsl],
                            wo_sb[:, cc, jsl],
                            start=(cc == 0),
                            stop=(cc == 3),
                        )
                    ot = otpool.tile([P, TCH], BF16, tag="ot", name="ot")
                    nc.vector.tensor_copy(ot[:], op[:])
                    nc.sync.dma_start(outp.ap()[ttsl, jsl], ot[:])


def _build(S=S_FULL):
    key = ("nc", S)
    if key in _CACHE:
        return _CACHE[key]
    nc = bacc.Bacc("TRN2", target_bir_lowering=False, debug=False, num_devices=8)
    xT = nc.dram_tensor("xT", [D, S], BF16, kind="ExternalInput")
    wqkT = nc.dram_tensor("wqkT", [D, FQK], BF16, kind="ExternalInput")
    wvT = nc.dram_tensor("wvT", [D, FV], BF16, kind="ExternalInput")
    woT = nc.dram_tensor("woT", [FV, D], BF16, kind="ExternalInput")
    cosF = nc.dram_tensor("cosF", [P, S], BF16, kind="ExternalInput")
    sinFpm = nc.dram_tensor("sinFpm", [P, S], BF16, kind="ExternalInput")
    mtri = nc.dram_tensor("mtri", [P, P], BF16, kind="ExternalInput")
    ident = nc.dram_tensor("ident", [P, P], BF16, kind="ExternalInput")
    ones2 = nc.dram_tensor("ones2", [1, 2 * P], F32R, kind="ExternalInput")
    onesv = nc.dram_tensor(
        "onesv", [P, S // P, NH_CORE, 1], BF16, kind="ExternalInput"
    )
    outp = nc.dram_tensor("outp", [S, D], BF16, kind="ExternalOutput")
    with tile.TileContext(nc) as tc:
        _emit(nc, tc, S, xT, wqkT, wvT, woT, cosF, sinFpm, mtri, ident, ones2, onesv, outp)
    nc.compile()
    _CACHE[key] = nc
    return nc


def host_inputs(x, wqkv, wo, token_positions, S=S_FULL):
    """Build the 8 per-core input maps (host-side sharding / layout prep)."""
    x = np.asarray(x, dtype=np.float32)
    wqkv = np.asarray(wqkv, dtype=np.float32)
    wo = np.asarray(wo, dtype=np.float32)
    pos = np.asarray(token_positions).astype(np.float32)

    d_model = x.shape[2]
    wq, wk, wv = wqkv[0:d_model], wqkv[d_model : 2 * d_model], wqkv[2 * d_model :]

    inv = np.float32(ROPE_THETA) ** (
        -np.arange(0, DH, 2, dtype=np.float32) / np.float32(DH)
    )  # [32]
    ang = pos[None, :] * inv[:, None]  # [32, S]
    cos32 = np.cos(ang).astype(np.float32)
    sin32 = np.sin(ang).astype(np.float32)

    import ml_dtypes

    cosF = np.tile(cos32, (4, 1)).astype(ml_dtypes.bfloat16)  # [128, S]
    sinFpm = np.tile(np.concatenate([sin32, -sin32], axis=0), (2, 1)).astype(
        ml_dtypes.bfloat16
    )  # [128, S]

    a = np.arange(P)
    mtri = np.where(a[:, None] > a[None, :], np.float32(NEG), np.float32(0.0))
    mtri = mtri.astype(ml_dtypes.bfloat16)
    ident = np.eye(P, dtype=ml_dtypes.bfloat16)
    S = x.shape[1]
    ones2 = np.zeros((1, 2 * P), np.float32)
    ones2[0, 0:64] = 1.0
    ones2[0, P + 64 : 2 * P] = 1.0
    onesv = np.ones((P, S // P, NH_CORE, 1), ml_dtypes.bfloat16)

    perm64 = np.concatenate([np.arange(0, DH, 2), np.arange(1, DH, 2)])

    in_maps = []
    for ci in range(8):
        bi, hg = divmod(ci, 2)
        xT = np.ascontiguousarray(x[bi].T)
        rows = []
        for blk in (wq, wk):
            for h in range(hg * NH_CORE, (hg + 1) * NH_CORE):
                rows.append(blk[h * DH : (h + 1) * DH][perm64])
        wqkT = np.ascontiguousarray(np.concatenate(rows, axis=0).T)
        wvT = np.ascontiguousarray(wv[hg * FV : (hg + 1) * FV].T)
        woT = np.ascontiguousarray(wo[:, hg * FV : (hg + 1) * FV].T)
        xT = xT.astype(ml_dtypes.bfloat16)
        wqkT = wqkT.astype(ml_dtypes.bfloat16)
        wvT = wvT.astype(ml_dtypes.bfloat16)
        woT = woT.astype(ml_dtypes.bfloat16)
        in_maps.append(
            {
                "xT": xT,
                "wqkT": wqkT,
                "wvT": wvT,
                "woT": woT,
                "cosF": cosF,
                "sinFpm": sinFpm,
                "mtri": mtri,
                "ident": ident,
                "ones2": ones2,
                "onesv": onesv,
            }
        )
    return in_maps


def _install_ntff_hook():
    """Recreate the antenv.axon_hooks NTFF profile hook this image lacks
    (same ctypes shim trn_agent_boot would register). Dev/profiling only."""
    import contextlib
    import ctypes
    import os
    import types

    try:
        import antenv.axon_hooks  # noqa: F401

        return
    except ImportError:
        pass
    so_path = "/opt/axon/libaxon_pjrt.so"
    if not os.path.exists(so_path):
        return
    lib = ctypes.CDLL(so_path)
    if not hasattr(lib, "axon_start_nrt_profile"):
        return
    lib.axon_start_nrt_profile.argtypes = [
        ctypes.POINTER(ctypes.c_int64),
        ctypes.c_size_t,
    ]
    lib.axon_start_nrt_profile.restype = ctypes.c_int64
    lib.axon_stop_nrt_profile.argtypes = [ctypes.c_char_p]
    lib.axon_stop_nrt_profile.restype = ctypes.c_int64

    @contextlib.contextmanager
    def _hook(output_dir, device_ids):
        import jax

        jax.devices()
        if device_ids:
            ids = (ctypes.c_int64 * len(device_ids))(*device_ids)
            rc = lib.axon_start_nrt_profile(ids, len(device_ids))
        else:
            rc = lib.axon_start_nrt_profile(None, 0)
        if rc != 0:
            raise RuntimeError(f"axon_start_nrt_profile rc={rc}")
        try:
            yield
        finally:
            n = lib.axon_stop_nrt_profile(str(output_dir).encode())
            if n < 0:
                raise RuntimeError(f"axon_stop_nrt_profile rc={n}")

    import antenv
    from concourse import bass_utils as _bu

    _bu.upload_artifacts = lambda d: d  # no bucket access in this container
    mod = types.ModuleType("antenv.axon_hooks")
    mod.get_axon_ntff_profile_hook = lambda: _hook
    mod.set_axon_ntff_profile_hook = lambda h: None
    sys.modules["antenv.axon_hooks"] = mod
    antenv.axon_hooks = mod


def kernel(x, wqkv, wo, token_positions, trace=False):
    if trace:
        _install_ntff_hook()
    nc = _build()
    in_maps = host_inputs(x, wqkv, wo, token_positions)
    res = run_bass_kernel_spmd(nc, in_maps, core_ids=list(range(8)), trace=trace)
    parts = [np.asarray(res.results[ci]["outp"], dtype=np.float32) for ci in range(8)]
    out = np.stack([parts[2 * bi] + parts[2 * bi + 1] for bi in range(B)], axis=0)
    if trace:
        kernel.last_result = res
    return out


# revision 12
# speedup vs baseline: 1.4584x; 1.2665x over previous
"""Causal multi-head self-attention (b=4, s=2048, d_model=1024, 16 heads) on 8
Trainium2 NeuronCores.

Sharding: core c handles batch c//2 and head-group c%2 (8 of 16 heads):
  - wqkv row-split by head (tensor parallel), wo column-split by head.
  - Each core returns the partial output projection [s, d_model] for its head
    group; the host sums the two partials of each batch while unsharding (the
    pairwise all-reduce of the TP split).

v2: t-chunk-major software pipeline. The kernel runs 4 stages, one per
512-token chunk. Each stage does, for its chunk t:
  A) Q/K projection + fused RoPE for all 4 head pairs (PSUM -> bf16 CAST,
     two 2x-mode bf16 tensor_tensor multiplies with bf16 cos/sin tables,
     SWDGE swap-add DMA for the rotate-half),
  B) V projection for the chunk (66-stride vbuf layout so the PSUM->SBUF
     CAST runs in 2x mode; ones column at 64 for free softmax denominators),
  C) causal attention for q-chunk t against all k-tiles <= t for all 4 head
     pairs (row-tiled score matmuls K=64x2, mask via identity@mtri accumulate,
     exp on ScalarE with column trimming, AV with M=65 ones-column lhsT,
     denominator broadcast via K=2 block-diag ones matmul + DVE reciprocal),
  D) output projection for the chunk's rows (contraction over all 4 head
     pairs' yT), bf16 staging, DMA out.
This keeps ScalarE's exp stream (the #2 engine, ~150us) overlapped with
TensorE work for the whole kernel instead of only the middle, starts matmuls
within ~5us of launch, and eliminates the phase-B/D serial head and tail of
v1. x is loaded once per stage (v1 loaded it twice).
"""

import sys

if "/opt/trn_rl_repo" not in sys.path:
    sys.path.insert(0, "/opt/trn_rl_repo")

from contextlib import ExitStack

import numpy as np

import concourse.bass as bass  # noqa: F401
import concourse.tile as tile
from concourse import bacc, mybir
from concourse.bass_utils import run_bass_kernel_spmd

F32 = mybir.dt.float32
F32R = mybir.dt.float32r
BF16 = mybir.dt.bfloat16
EXP = mybir.ActivationFunctionType.Exp
MULT = mybir.AluOpType.mult
ADD = mybir.AluOpType.add

# Problem constants
B, S_FULL, D = 4, 2048, 1024
NH_CORE = 8      # heads per core
DH = 64          # head dim
FQK = 1024       # Q+K features per core
FV = 512         # V features per core
P = 128
TCH = 512        # q/t chunk size
VST = 66         # vbuf per-head stride (64 V dims + ones col + pad for 4B align)
NEG = -1.0e30
ROPE_THETA = 10000.0
SCALE = 1.0 / 8.0  # 1/sqrt(DH)

_CACHE = {}


def _emit(nc, tc, S, xT, wqkT, wvT, woT, cosF, sinFpm, mtri, ident, ones2, outp):
    n_st = S // TCH          # stages
    n_hp = NH_CORE // 2
    mm = nc.tensor.matmul

    xT_r = xT.ap().rearrange("(eo p) t -> p eo t", p=P)
    wqk_r = wqkT.ap().rearrange("(eo p) f -> p eo f", p=P)
    wv_r = wvT.ap().rearrange("(eo p) f -> p eo f", p=P)
    wo_r = woT.ap().rearrange("(co p) j -> p co j", p=P)

    with ExitStack() as ctx:
        # ---------- persistent SBUF ----------
        persist = ctx.enter_context(tc.tile_pool(name="persist", bufs=1))
        wqk_sb = persist.tile([P, 8, FQK], BF16, tag="wqk", name="wqk_sb")
        wv_sb = persist.tile([P, 8, FV], BF16, tag="wv", name="wv_sb")
        wo_sb = persist.tile([P, 4, D], BF16, tag="wo", name="wo_sb")
        qkT = [persist.tile([P, S], BF16, tag=f"qkT{ft}", name=f"qkT{ft}") for ft in range(8)]
        vbuf = persist.tile([P, S // P, NH_CORE, VST], BF16, tag="vbuf", name="vbuf")
        yT = [persist.tile([P, S], BF16, tag=f"yT{hp}", name=f"yT{hp}") for hp in range(n_hp)]
        ident_sb = persist.tile([P, P], BF16, tag="ident", name="ident_sb")
        mtri_sb = persist.tile([P, P], BF16, tag="mtri", name="mtri_sb")
        ones2_sb = persist.tile([1, 2 * P], F32R, tag="ones2", name="ones2_sb")

        # streamed per-stage inputs
        xpool = ctx.enter_context(tc.tile_pool(name="xchunk", bufs=2))
        cspool = ctx.enter_context(tc.tile_pool(name="costab", bufs=2))
        bfpool = ctx.enter_context(tc.tile_pool(name="qkbf", bufs=3))
        btpool = ctx.enter_context(tc.tile_pool(name="btmp", bufs=3))
        epool = ctx.enter_context(tc.tile_pool(name="expS", bufs=5))
        dpool = ctx.enter_context(tc.tile_pool(name="denst", bufs=2))
        rpool = ctx.enter_context(tc.tile_pool(name="recb", bufs=2))
        otpool = ctx.enter_context(tc.tile_pool(name="outsb", bufs=3))

        # PSUM: 4 + 2 + 2 = 8 banks
        s_ps = ctx.enter_context(tc.tile_pool(name="s_psum", bufs=2, space="PSUM"))
        av_ps = ctx.enter_context(tc.tile_pool(name="av_psum", bufs=1, space="PSUM"))
        m_ps = ctx.enter_context(tc.tile_pool(name="m_psum", bufs=2, space="PSUM"))

        # ---------- initial loads ----------
        # stage-0 activations first (smallest deps of the first matmuls),
        # weights f-half-sliced and interleaved across the two HWDGE queues
        def load_stage_inputs(s):
            tsl = slice(s * TCH, (s + 1) * TCH)
            xch = xpool.tile([P, 8, TCH], BF16, tag="xch", name="xch")
            nc.sync.dma_start(xch[:], xT_r[:, :, tsl])
            cos_ch = cspool.tile([P, TCH], BF16, tag="cos", name="cos")
            sin_ch = cspool.tile([P, TCH], BF16, tag="sin", name="sin")
            nc.scalar.dma_start(cos_ch[:], cosF.ap()[:, tsl])
            nc.scalar.dma_start(sin_ch[:], sinFpm.ap()[:, tsl])
            return xch, cos_ch, sin_ch

        xin = load_stage_inputs(0)
        for fh in range(2):
            fsl = slice(fh * 512, (fh + 1) * 512)
            for ec in range(8):
                eng = nc.sync if ec % 2 == 0 else nc.scalar
                eng.dma_start(wqk_sb[:, ec, fsl], wqk_r[:, ec, fsl])
        nc.sync.dma_start(ident_sb[:], ident.ap()[:, :])
        nc.sync.dma_start(mtri_sb[:], mtri.ap()[:, :])
        nc.scalar.dma_start(ones2_sb[:], ones2.ap()[:, :])
        nc.gpsimd.memset(vbuf[:, :, :, DH : DH + 1], 1.0)
        for ec in range(8):
            eng = nc.scalar if ec % 2 == 0 else nc.sync
            eng.dma_start(wv_sb[:, ec, :], wv_r[:, ec, :])
        for cc in range(4):
            eng = nc.sync if cc % 2 == 0 else nc.scalar
            eng.dma_start(wo_sb[:, cc, :], wo_r[:, cc, :])

        # ---------- per-stage work units ----------
        def a_unit(s, xch, cos_ch, sin_ch, ft):
            # Q/K projection + fused RoPE for f-tile ft, chunk s
            tsl = slice(s * TCH, (s + 1) * TCH)
            qkps = m_ps.tile([P, TCH], F32, tag="m", name="qkps")
            for ec in range(8):
                mm(
                    qkps[:],
                    wqk_sb[:, ec, ft * P : (ft + 1) * P],
                    xch[:, ec, :],
                    start=(ec == 0),
                    stop=(ec == 7),
                )
            qk_bf = bfpool.tile([P, TCH], BF16, tag="qkbf", name="qkbf")
            nc.vector.tensor_copy(qk_bf[:], qkps[:])
            cp = bfpool.tile([P, TCH], BF16, tag="cp", name="cp")
            nc.vector.tensor_tensor(cp[:], qk_bf[:], cos_ch[:], MULT)
            bt = btpool.tile([P, TCH], BF16, tag="bt", name="bt")
            nc.vector.tensor_tensor(bt[:], qk_bf[:], sin_ch[:], MULT)
            # rotate-half: HWDGE 32-row swapped copy, then one 2x bf16 add
            bs = btpool.tile([P, TCH], BF16, tag="bs", name="bs")
            for blk in range(4):
                a = blk * 32
                c2 = a ^ 32
                nc.sync.dma_start(bs[c2 : c2 + 32, :], bt[a : a + 32, :])
            nc.vector.tensor_tensor(qkT[ft][:, tsl], cp[:], bs[:], ADD)

        def b_unit(s, xch, tti):
            # V projection for t-tile tti of chunk s
            kt = s * (TCH // P) + tti
            vps = m_ps.tile([P, FV], F32, tag="m", name="vps")
            for ec in range(8):
                mm(
                    vps[:],
                    xch[:, ec, tti * P : (tti + 1) * P],
                    wv_sb[:, ec, :],
                    start=(ec == 0),
                    stop=(ec == 7),
                )
            nc.vector.tensor_copy(vbuf[:, kt, :, 0:DH], vps[:])

        def d_unit(s, tti, jc):
            # output projection for t-tile tti of chunk s, j-half jc
            ttsl = slice(s * TCH + tti * P, s * TCH + (tti + 1) * P)
            jsl = slice(jc * TCH, (jc + 1) * TCH)
            op = m_ps.tile([P, TCH], F32, tag="m", name="op")
            for cc in range(4):
                mm(
                    op[:],
                    yT[cc][:, ttsl],
                    wo_sb[:, cc, jsl],
                    start=(cc == 0),
                    stop=(cc == 3),
                )
            ot = otpool.tile([P, TCH], BF16, tag="ot", name="ot")
            nc.vector.tensor_copy(ot[:], op[:])
            nc.sync.dma_start(outp.ap()[ttsl, jsl], ot[:])

        def ab_units(s, xin_s):
            xch, cos_ch, sin_ch = xin_s
            for hp in range(n_hp):
                yield lambda ft=hp: a_unit(s, xch, cos_ch, sin_ch, ft)
                yield lambda ft=4 + hp: a_unit(s, xch, cos_ch, sin_ch, ft)
                yield lambda tti=hp: b_unit(s, xch, tti)

        def d_units(s):
            for tti in range(TCH // P):
                for jc in range(2):
                    yield lambda tti=tti, jc=jc: d_unit(s, tti, jc)

        # stage 0 prologue: project chunk 0 outright
        for u in ab_units(0, xin):
            u()

        for s in range(n_st):
            tsl = slice(s * TCH, (s + 1) * TCH)
            if s + 1 < n_st:
                xin = load_stage_inputs(s + 1)

            # fillers: next chunk's projections + previous chunk's output
            # projection, interleaved between C's exp-paced k-tiles so the
            # TensorE queue never stalls on a ScalarE wait (keeps HAM warm)
            fillers = []
            if s + 1 < n_st:
                fillers.extend(ab_units(s + 1, xin))
            if s >= 1:
                fillers.extend(d_units(s - 1))
            nkt = (TCH // P) * (s + 1)
            n_slots = n_hp * nkt
            acc = 0.0
            rate = len(fillers) / n_slots if n_slots else 0.0

            # ---- C: attention, q-chunk s vs k-tiles 0..4s+3 ----
            for hp in range(n_hp):
                qt = qkT[hp]
                ktt = qkT[4 + hp]
                h0, h1 = 2 * hp, 2 * hp + 1
                avp = av_ps.tile([DH + 1, 2 * TCH], F32, tag="avp", name="avp")
                for ki in range(nkt):
                    acc += rate
                    while acc >= 1.0 and fillers:
                        fillers.pop(0)()
                        acc -= 1.0
                    ksl = slice(ki * P, (ki + 1) * P)
                    diag = ki >= (TCH // P) * s
                    j = ki - (TCH // P) * s
                    off = j * P if diag else 0
                    qsl = slice(s * TCH + off, (s + 1) * TCH)
                    sp = s_ps.tile([P, 2 * TCH], F32, tag="sp", name="sp")
                    mm(
                        sp[:, off:TCH],
                        ktt[0:64, ksl],
                        qt[0:64, qsl],
                        start=True,
                        stop=True,
                    )
                    mm(
                        sp[:, TCH + off : 2 * TCH],
                        ktt[64:128, ksl],
                        qt[64:128, qsl],
                        start=True,
                        stop=True,
                    )
                    if diag:
                        mm(
                            sp[:, off : off + P],
                            ident_sb[:],
                            mtri_sb[:],
                            start=False,
                            stop=True,
                            skip_group_check=True,
                        )
                        mm(
                            sp[:, TCH + off : TCH + off + P],
                            ident_sb[:],
                            mtri_sb[:],
                            start=False,
                            stop=True,
                            skip_group_check=True,
                        )
                    e = epool.tile([P, 2 * TCH], BF16, tag="e", name="e")
                    sp3 = sp[:].rearrange("p (h q) -> p h q", h=2)
                    e3 = e[:].rearrange("p (h q) -> p h q", h=2)
                    nc.scalar.activation(
                        e3[:, :, off:], sp3[:, :, off:], EXP, scale=SCALE
                    )
                    mm(
                        avp[:, off:TCH],
                        vbuf[:, ki, h0, 0 : DH + 1],
                        e[:, off:TCH],
                        start=(ki == 0),
                        stop=(ki == nkt - 1),
                        skip_group_check=True,
                    )
                    mm(
                        avp[:, TCH + off : 2 * TCH],
                        vbuf[:, ki, h1, 0 : DH + 1],
                        e[:, TCH + off : 2 * TCH],
                        start=(ki == 0),
                        stop=(ki == nkt - 1),
                        skip_group_check=True,
                    )
                # denominators (avp row 64) -> two K=1 masked matmuls broadcast
                # head0's denoms to partitions 0:64 and head1's to 64:128
                denf = dpool.tile([1, 2 * TCH], F32R, tag="denf", name="denf")
                nc.vector.tensor_copy(denf[:], avp[DH : DH + 1, :])
                rb = m_ps.tile([P, TCH], F32, tag="m", name="rb")
                mm(
                    rb[:],
                    ones2_sb[0:1, 0:P],
                    denf[0:1, 0:TCH],
                    start=True,
                    stop=False,
                )
                mm(
                    rb[:],
                    ones2_sb[0:1, P : 2 * P],
                    denf[0:1, TCH : 2 * TCH],
                    start=False,
                    stop=True,
                )
                rec = rpool.tile([P, TCH], F32, tag="rec", name="rec")
                rscr = rpool.tile([P, TCH], F32, tag="rscr", name="rscr")
                nc.vector.reciprocal_approx_accurate(rec[:], rb[:], rscr[:])
                nc.vector.tensor_tensor(
                    yT[hp][0:64, tsl], avp[0:DH, 0:TCH], rec[0:64, :], MULT
                )
                nc.vector.tensor_tensor(
                    yT[hp][64:128, tsl],
                    avp[0:DH, TCH : 2 * TCH],
                    rec[64:128, :],
                    MULT,
                )

            # drain any leftover fillers for this stage
            for u in fillers:
                u()

        # epilogue: output projection for the final chunk
        for u in d_units(n_st - 1):
            u()


def _build(S=S_FULL):
    key = ("nc", S)
    if key in _CACHE:
        return _CACHE[key]
    nc = bacc.Bacc("TRN2", target_bir_lowering=False, debug=False, num_devices=8)
    xT = nc.dram_tensor("xT", [D, S], BF16, kind="ExternalInput")
    wqkT = nc.dram_tensor("wqkT", [D, FQK], BF16, kind="ExternalInput")
    wvT = nc.dram_tensor("wvT", [D, FV], BF16, kind="ExternalInput")
    woT = nc.dram_tensor("woT", [FV, D], BF16, kind="ExternalInput")
    cosF = nc.dram_tensor("cosF", [P, S], BF16, kind="ExternalInput")
    sinFpm = nc.dram_tensor("sinFpm", [P, S], BF16, kind="ExternalInput")
    mtri = nc.dram_tensor("mtri", [P, P], BF16, kind="ExternalInput")
    ident = nc.dram_tensor("ident", [P, P], BF16, kind="ExternalInput")
    ones2 = nc.dram_tensor("ones2", [1, 2 * P], F32R, kind="ExternalInput")
    outp = nc.dram_tensor("outp", [S, D], BF16, kind="ExternalOutput")
    with tile.TileContext(nc) as tc:
        _emit(nc, tc, S, xT, wqkT, wvT, woT, cosF, sinFpm, mtri, ident, ones2, outp)
    nc.compile()
    _CACHE[key] = nc
    return nc


def host_inputs(x, wqkv, wo, token_positions, S=S_FULL):
    """Build the 8 per-core input maps (host-side sharding / layout prep)."""
    x = np.asarray(x, dtype=np.float32)
    wqkv = np.asarray(wqkv, dtype=np.float32)
    wo = np.asarray(wo, dtype=np.float32)
    pos = np.asarray(token_positions).astype(np.float32)

    d_model = x.shape[2]
    wq, wk, wv = wqkv[0:d_model], wqkv[d_model : 2 * d_model], wqkv[2 * d_model :]

    inv = np.float32(ROPE_THETA) ** (
        -np.arange(0, DH, 2, dtype=np.float32) / np.float32(DH)
    )  # [32]
    ang = pos[None, :] * inv[:, None]  # [32, S]
    cos32 = np.cos(ang).astype(np.float32)
    sin32 = np.sin(ang).astype(np.float32)

    import ml_dtypes

    cosF = np.tile(cos32, (4, 1)).astype(ml_dtypes.bfloat16)  # [128, S]
    sinFpm = np.tile(np.concatenate([sin32, -sin32], axis=0), (2, 1)).astype(
        ml_dtypes.bfloat16
    )  # [128, S]

    a = np.arange(P)
    mtri = np.where(a[:, None] > a[None, :], np.float32(NEG), np.float32(0.0))
    mtri = mtri.astype(ml_dtypes.bfloat16)
    ident = np.eye(P, dtype=ml_dtypes.bfloat16)
    S = x.shape[1]
    ones2 = np.zeros((1, 2 * P), np.float32)
    ones2[0, 0:64] = 1.0
    ones2[0, P + 64 : 2 * P] = 1.0

    perm64 = np.concatenate([np.arange(0, DH, 2), np.arange(1, DH, 2)])

    in_maps = []
    for ci in range(8):
        bi, hg = divmod(ci, 2)
        xT = np.ascontiguousarray(x[bi].T)
        rows = []
        for blk in (wq, wk):
            for h in range(hg * NH_CORE, (hg + 1) * NH_CORE):
                rows.append(blk[h * DH : (h + 1) * DH][perm64])
        wqkT = np.ascontiguousarray(np.concatenate(rows, axis=0).T)
        wvT = np.ascontiguousarray(wv[hg * FV : (hg + 1) * FV].T)
        woT = np.ascontiguousarray(wo[:, hg * FV : (hg + 1) * FV].T)
        xT = xT.astype(ml_dtypes.bfloat16)
        wqkT = wqkT.astype(ml_dtypes.bfloat16)
        wvT = wvT.astype(ml_dtypes.bfloat16)
        woT = woT.astype(ml_dtypes.bfloat16)
        in_maps.append(
            {
                "xT": xT,
                "wqkT": wqkT,
                "wvT": wvT,
                "woT": woT,
                "cosF": cosF,
                "sinFpm": sinFpm,
                "mtri": mtri,
                "ident": ident,
                "ones2": ones2,
            }
        )
    return in_maps


def _install_ntff_hook():
    """Recreate the antenv.axon_hooks NTFF profile hook this image lacks
    (same ctypes shim trn_agent_boot would register). Dev/profiling only."""
    import contextlib
    import ctypes
    import os
    import types

    try:
        import antenv.axon_hooks  # noqa: F401

        return
    except ImportError:
        pass
    so_path = "/opt/axon/libaxon_pjrt.so"
    if not os.path.exists(so_path):
        return
    lib = ctypes.CDLL(so_path)
    if not hasattr(lib, "axon_start_nrt_profile"):
        return
    lib.axon_start_nrt_profile.argtypes = [
        ctypes.POINTER(ctypes.c_int64),
        ctypes.c_size_t,
    ]
    lib.axon_start_nrt_profile.restype = ctypes.c_int64
    lib.axon_stop_nrt_profile.argtypes = [ctypes.c_char_p]
    lib.axon_stop_nrt_profile.restype = ctypes.c_int64

    @contextlib.contextmanager
    def _hook(output_dir, device_ids):
        import jax

        jax.devices()
        if device_ids:
            ids = (ctypes.c_int64 * len(device_ids))(*device_ids)
            rc = lib.axon_start_nrt_profile(ids, len(device_ids))
        else:
            rc = lib.axon_start_nrt_profile(None, 0)
        if rc != 0:
            raise RuntimeError(f"axon_start_nrt_profile rc={rc}")
        try:
            yield
        finally:
            n = lib.axon_stop_nrt_profile(str(output_dir).encode())
            if n < 0:
                raise RuntimeError(f"axon_stop_nrt_profile rc={n}")

    import antenv
    from concourse import bass_utils as _bu

    _bu.upload_artifacts = lambda d: d  # no bucket access in this container
    mod = types.ModuleType("antenv.axon_hooks")
    mod.get_axon_ntff_profile_hook = lambda: _hook
    mod.set_axon_ntff_profile_hook = lambda h: None
    sys.modules["antenv.axon_hooks"] = mod
    antenv.axon_hooks = mod


def kernel(x, wqkv, wo, token_positions, trace=False):
    if trace:
        _install_ntff_hook()
    nc = _build()
    in_maps = host_inputs(x, wqkv, wo, token_positions)
    res = run_bass_kernel_spmd(nc, in_maps, core_ids=list(range(8)), trace=trace)
    parts = [np.asarray(res.results[ci]["outp"], dtype=np.float32) for ci in range(8)]
    out = np.stack([parts[2 * bi] + parts[2 * bi + 1] for bi in range(B)], axis=0)
    if trace:
        kernel.last_result = res
    return out
